# revision 17
# baseline (speedup 1.0000x reference)
"""Trainium2 Bass kernel: causal multi-head attention (B=4,S=2048,D=1024,H=16).

Sharding (8 cores, pair-wise AllGather): core c -> batch b=c//2,
head-half hh=c%2 (heads hh*8..hh*8+7).  Each core projects K/V/Q for its
8 heads over the FULL sequence (no duplicated K/V work), runs causal
attention for all 2048 queries (two interleaved q-passes of 8 x 128-row
slots per head), exchanges each completed head-pair's concat with its
batch peer via a pair-wise AllGather overlapped with the remaining
attention, then computes fc_out for all 2048 rows from the gathered
full concat; the host keeps each core's contiguous sequence half.

Device pipeline per core (all matmuls bf16, f32 accumulation), organized
to keep the tensor engine continuously busy:

  A: x^T via PE transposes (casts/evacuations alternate ScalarE/VectorE,
     DMAs spread over the sync+scalar queues), all weight tensors loaded
     via casting gpsimd DMAs (f32->bf16 in the DMA), then a dense
     V-projection pass over all 16 s-tiles with pair-0 K^T/Q^T blocks
     interleaved.
  C: per (head, q-pass), per k-tile: scores^T into a 2-bank PSUM tile
     (double buffered), ONE exp ACTIVATE per k-tile (narrow adjacent
     k-tiles are paired into a single strided ACTIVATE), 0/1 mask
     multiply on mixed tiles only, out^T accumulation per 512-col group
     with ones-augmented V (row 64 = softmax denominator).  Softmax
     normalization is split: the PSUM-side prep is emitted as soon as a
     group's accumulation completes, while the tensor-engine finalize
     for the last group is deferred into the next (head, pass) stream.
     K^T/Q^T projection chunks for pair g+1 (and the Wo casting DMAs)
     are interleaved into pair g's attention stream.
  D: as soon as pair g's attention (4 head/pass units) completes, its
     concat tile cat[g] (bf16, 512KB) is DMAd to DRAM and a pair-wise
     AllGather fires, hidden under the remaining attention work (only
     pair 3's gather is partially exposed, covered by ordering its
     fc_out chunks last).  fc_out reads ONLY the gathered concat (both
     ranks' heads, global order) and computes all 16 q-tiles with the
     full Wo; the host keeps each core's sequence half.

The program is specialized at build time to the mask's block structure
per q-pass (skip all-zero blocks / skip masking on all-ones blocks);
this is computed from the actual mask input, so it stays correct for
any mask.
"""

import os
import numpy as np
import ml_dtypes

import concourse.bass as bass
import concourse.mybir as mybir
import concourse.tile as tile
from concourse import bacc
from concourse.bass_utils import run_bass_kernel_spmd

B, S, D, H, HD = 4, 2048, 1024, 16, 64
N_CORES = 8
ST = 128               # tile edge (partition size)
NKT = S // ST          # 16 key tiles
NJ = 8                 # q slots per pass (8*128 = 1024 rows)
NP = 2                 # q passes per head (slot j of pass p = abs tile 2j+p)
HL = H // 2            # 8 local heads per core
NDC = D // ST          # 8 contraction chunks
NG = HL // 2           # 4 local head pairs (2 heads per 128 partitions)
NSG = S // 512         # 4 s-groups of 512 cols for K^T/Q^T projections
NB = NJ // 4           # 2 x 512-col output groups of slots per pass
NWOC = (HL * HD) // ST  # 4 local Wo contraction chunks

F32 = mybir.dt.float32
BF16 = mybir.dt.bfloat16

CC_GROUPS = [[0, 1], [2, 3], [4, 5], [6, 7]]


def _classify(mask: np.ndarray):
    """Block structure of the mask per q-pass.

    Returns (cls[NP][NJ][NKT] in {0 skip,1 full,2 mixed}, mixed list of
    (p,j,k), mixed->dedup-index map, number of distinct mask tiles).
    """
    cls = np.zeros((NP, NJ, NKT), dtype=int)
    for p in range(NP):
        for j in range(NJ):
            t = 2 * j + p
            for k in range(NKT):
                blk = mask[t * ST:(t + 1) * ST, k * ST:(k + 1) * ST]
                if (blk != 0).all():
                    cls[p, j, k] = 1
                elif (blk == 0).all():
                    cls[p, j, k] = 0
                else:
                    cls[p, j, k] = 2
            # close interior holes so the computed k-range is contiguous
            nz = np.nonzero(cls[p, j])[0]
            if len(nz):
                for k in range(nz[0], nz[-1] + 1):
                    if cls[p, j, k] == 0:
                        cls[p, j, k] = 2
    mixed = [(p, j, k) for p in range(NP) for j in range(NJ)
             for k in range(NKT) if cls[p, j, k] == 2]
    dedup = {}
    midx = {}
    for (p, j, k) in mixed:
        t = 2 * j + p
        key = mask[t * ST:(t + 1) * ST, k * ST:(k + 1) * ST].tobytes()
        if key not in dedup:
            dedup[key] = len(dedup)
        midx[(p, j, k)] = dedup[key]
    return cls, mixed, midx, max(len(dedup), 1)


def _build(cls, mixed, mixed_idx, n_maskt):
    """Build the (uniform, SPMD) Bass program for one core's shard."""
    nc = bacc.Bacc("TRN2", target_bir_lowering=False, debug=False,
                   num_devices=N_CORES)

    x_d = nc.dram_tensor("x", [S, D], BF16, kind="ExternalInput")
    wq_d = nc.dram_tensor("wq", [HL, D, HD], BF16, kind="ExternalInput")
    wk_d = nc.dram_tensor("wk", [HL, D, HD], BF16, kind="ExternalInput")
    wv_d = nc.dram_tensor("wv", [HL, D, HD], BF16, kind="ExternalInput")
    wo_d = nc.dram_tensor("wo", [D, D], BF16, kind="ExternalInput")
    bq_d = nc.dram_tensor("bq", [HL, HD], F32, kind="ExternalInput")
    bk_d = nc.dram_tensor("bk", [HL, HD], F32, kind="ExternalInput")
    bv_d = nc.dram_tensor("bv", [HL, HD], F32, kind="ExternalInput")
    bo_d = nc.dram_tensor("bo", [D], F32, kind="ExternalInput")
    mt_d = nc.dram_tensor("maskt", [n_maskt, ST, ST], BF16, kind="ExternalInput")
    out_d = nc.dram_tensor("out", [S, D], F32, kind="ExternalOutput")

    # per-pass mask/block structure
    slots_k = [[[j for j in range(NJ) if cls[p, j, k]] for k in range(NKT)]
               for p in range(NP)]
    kfirst = [{} for _ in range(NP)]
    klast = [{} for _ in range(NP)]
    for p in range(NP):
        for j in range(NJ):
            ks = [k for k in range(NKT) if cls[p, j, k]]
            if ks:
                kfirst[p][j], klast[p][j] = ks[0], ks[-1]
    bank_slots = [[[j for j in range(4 * b_, 4 * b_ + 4) if j in kfirst[p]]
                   for b_ in range(NB)] for p in range(NP)]
    bklast = [{b_: max(klast[p][j] for j in bank_slots[p][b_])
               for b_ in range(NB) if bank_slots[p][b_]} for p in range(NP)]
    bank_fast = [{b_: len({kfirst[p][j] for j in bank_slots[p][b_]}) == 1
                  for b_ in range(NB) if bank_slots[p][b_]}
                 for p in range(NP)]

    from concourse.masks import make_identity

    with tile.TileContext(nc) as tc:
        with (
            tc.tile_pool(name="persist", bufs=1) as pp,
            tc.tile_pool(name="dram", bufs=1, space="DRAM") as dram,
        ):
            # ---- persistent SBUF tensors -------------------------------
            kt_t = [pp.tile([ST, S], BF16, name=f"ktg{g}", tag=f"ktg{g}")
                    for g in range(NG)]
            qt_t = [pp.tile([ST, S], BF16, name=f"qtg{g}", tag=f"qtg{g}")
                    for g in range(NG)]
            vb = pp.tile([ST, NKT, HL, HD + 1], BF16, name="vb", tag="vb")
            cat = [pp.tile([ST, S], BF16, name=f"catg{g}", tag=f"catg{g}")
                   for g in range(NG)]
            # gathered concat: catx[g][:, r, :] = rank r's pair g
            catx = [pp.tile([ST, 2, S], BF16, name=f"catx{g}",
                            tag=f"catx{g}") for g in range(NG)]
            ident = pp.tile([ST, ST], BF16, name="ident", tag="ident")
            ones1 = pp.tile([1, HD], BF16, name="ones1", tag="ones1")
            mtb = pp.tile([ST, max(n_maskt, 1), ST], BF16, name="mtb",
                          tag="mtb")

            # AllGather bounce buffers (one per local head pair)
            ag_in = [dram.tile([ST, S], BF16, name=f"agin{g}",
                               tag=f"agin{g}") for g in range(NG)]
            ag_out = [dram.tile([2, ST, S], BF16, name=f"agout{g}",
                                tag=f"agout{g}") for g in range(NG)]

            nc.vector.memset(vb[:, :, :, HD:HD + 1], 1.0)
            nc.vector.memset(ones1[:, :], 1.0)
            junk = pp.tile([1, 1], F32, name="junk", tag="junk")
            nc.vector.memset(junk[:, :], 0.0)
            make_identity(nc, ident[:, :])
            nc.scalar.dma_start(mtb[:, :, :],
                                mt_d.ap().rearrange("m p f -> p m f"))

            def load_bias_pair(pool, bias_d, name):
                # [128, NG] f32: partition = (h%2)*64+e, column = pair idx
                t = pool.tile([ST, NG], F32, name=name, tag=name, bufs=1)
                src = bias_d.ap()
                nc.scalar.dma_start(
                    t[:, :],
                    bass.AP(tensor=src.tensor, offset=src.offset,
                            ap=[[1, ST], [ST, NG]]))
                return t

            # x^T tiles (live until the last K/Q projection)
            xtp_cm = tc.tile_pool(name="xtp", bufs=1, side="right")
            xtp = xtp_cm.__enter__()
            xt_all = xtp.tile([ST, NDC, NKT, ST], BF16, name="xt_all",
                              tag="xt_all")

            # weight-pair destination pool (lives phase A .. attention)
            wp_cm = tc.tile_pool(name="wpair", bufs=2)
            wp = wp_cm.__enter__()

            bkp = load_bias_pair(pp, bk_d, "bkp")
            bqp = load_bias_pair(pp, bq_d, "bqp")

            def k_proj_block(wpr, g, sg, pool, pbufs=2):
                psk = pool.tile([ST, 512], F32, tag="psk", name="psk",
                                bufs=pbufs)
                for c in range(NDC):
                    nc.tensor.matmul(
                        psk[:, :], wpr[:, c, :],
                        xt_all[:, c, 4 * sg:4 * (sg + 1), :],
                        start=(c == 0), stop=(c == NDC - 1))
                nc.vector.tensor_scalar(
                    kt_t[g][:, sg * 512:(sg + 1) * 512],
                    psk[:, :], bkp[:, g:g + 1], None,
                    mybir.AluOpType.add)

            def q_proj_block(wpr, g, sg, pool, pbufs=2):
                psk = pool.tile([ST, 512], F32, tag="psk", name="psk",
                                bufs=pbufs)
                for c in range(NDC):
                    nc.tensor.matmul(
                        psk[:, :], wpr[:, c, :],
                        xt_all[:, c, 4 * sg:4 * (sg + 1), :],
                        start=(c == 0), stop=(c == NDC - 1))
                nc.vector.tensor_scalar(
                    qt_t[g][:, sg * 512:(sg + 1) * 512],
                    psk[:, :], bqp[:, g:g + 1], None,
                    mybir.AluOpType.add)

            def stage_pair_weights(w_d, g, tag):
                # casting gpsimd DMAs straight into the [128, NDC, 128]
                # stationary-pair layout (f32 -> bf16 in the DMA)
                wpr = wp.tile([ST, NDC, ST], BF16, name=f"{tag}{g}", tag=tag)
                for h2 in range(2):
                    src = w_d.ap()[2 * g + h2].rearrange(
                        "(c p) e -> p c e", p=ST)
                    nc.gpsimd.dma_start(
                        wpr[:, :, h2 * HD:(h2 + 1) * HD], src)
                return wpr

            # ---- phase A: x^T, then a dense V pass ---------------------
            with (
                tc.tile_pool(name="p1a", bufs=2) as p1a,
                tc.tile_pool(name="pv", bufs=1) as pv,
                tc.tile_pool(name="ppst", bufs=4, space="PSUM") as ppst,
                tc.tile_pool(name="ppsv", bufs=3, space="PSUM") as ppsv,
            ):
                wvb = pv.tile([ST, NDC, HL, HD], BF16, name="wvb", tag="wvb",
                              bufs=1)
                bvf = pv.tile([ST, HL, HD], F32, name="bvf", tag="bvf",
                              bufs=1)
                # gpsimd casting-DMA order matches consumption order:
                # all 8 V heads (dense pass), then pair-0 K/Q weights
                for hh in range(HL):
                    srcw = wv_d.ap()[hh].rearrange("(c p) e -> p c e", p=ST)
                    nc.gpsimd.dma_start(wvb[:, :, hh, :], srcw)
                _stage0 = (stage_pair_weights(wk_d, 0, "wkpr"),
                           stage_pair_weights(wq_d, 0, "wqpr"))

                evac_i = 0

                def transpose_tile(dst_all, sti, xsrc_ap):
                    nonlocal evac_i
                    xb = p1a.tile([ST, D], BF16, tag="xb", name="xb",
                                  bufs=6)
                    qeng = nc.sync if (evac_i // 8) % 2 == 0 else nc.scalar
                    qeng.dma_start(xb[:, :], xsrc_ap)
                    for c4 in range(NDC // 4):
                        pst4 = ppst.tile([ST, 4, ST], BF16, tag="pst",
                                         name="pst")
                        for i in range(4):
                            c = 4 * c4 + i
                            nc.tensor.transpose(
                                pst4[:, i, :], xb[:, c * ST:(c + 1) * ST],
                                ident[:, :])
                        if evac_i % 2 == 1:
                            nc.scalar.copy(
                                dst_all[:, 4 * c4:4 * c4 + 4, sti, :],
                                pst4[:, :, :])
                        else:
                            nc.vector.tensor_copy(
                                dst_all[:, 4 * c4:4 * c4 + 4, sti, :],
                                pst4[:, :, :])
                        evac_i += 1

                # preload the Exp activation table early
                nc.scalar.activation(junk[:, :], junk[:, :],
                                     mybir.ActivationFunctionType.Exp)
                # pair-0 K/Q chunks interleaved as their xt tiles land
                _p0 = []
                for sg in range(NSG):
                    _p0.append((k_proj_block, _stage0[0], sg, ppst))
                    _p0.append((q_proj_block, _stage0[1], sg, ppst))

                def vproj(st):
                    # dense V projection for s-tile st (all 8 heads)
                    psv = ppsv.tile([ST, HL * HD], F32, tag="psv",
                                    name="psv")
                    for c in range(NDC):
                        nc.tensor.matmul(
                            psv[:, :],
                            xt_all[:, c, st, :],
                            wvb[:, c, :, :],
                            start=(c == 0), stop=(c == NDC - 1))
                    nc.vector.tensor_add(
                        vb[:, st, :, 0:HD],
                        psv[:, :].rearrange("p (h e) -> p h e", h=HL),
                        bvf[:, :, :])
                    # K/Q s-group j//4 only needs xt tiles <= st
                    if st % 4 == 3:
                        for _ in range(2):
                            fn, w_, sg_, pl_ = _p0.pop(0)
                            fn(w_, 0, sg_, pl_, 1)

                # V-projection (and pair-0 K/Q) interleave into the
                # transpose stream so the PE has work while x loads
                for st in range(NKT):
                    transpose_tile(xt_all, st,
                                   x_d.ap()[st * ST:(st + 1) * ST, :])
                    if st == 0:
                        srcv = bv_d.ap()
                        nc.scalar.dma_start(
                            bvf[:, :, :],
                            bass.AP(tensor=srcv.tensor, offset=srcv.offset,
                                    ap=[[0, ST]] + list(srcv.ap)))
                    if st >= 1:
                        vproj(st - 1)
                vproj(NKT - 1)
                while _p0:
                    fn, w_, sg_, pl_ = _p0.pop(0)
                    fn(w_, 0, sg_, pl_, 1)

            # ---- phases B/C/D: projections + attention + fc_out --------
            # PSUM budget: psc 2x2 banks + po 2x1 banks + psk 2x1 = 8.
            p2s_cm = tc.tile_pool(name="p2s", bufs=2)
            p2s = p2s_cm.__enter__()
            wop_cm = tc.tile_pool(name="wop", bufs=1)
            wop = wop_cm.__enter__()
            wob = wop.tile([ST, NDC, D], BF16, name="wob", tag="wob")
            bob = wop.tile([ST, D], BF16, name="bob", tag="bob")
            bo_ap = bo_d.ap()
            nc.gpsimd.dma_start(
                bob[:, :],
                bass.AP(tensor=bo_ap.tensor, offset=bo_ap.offset,
                        ap=[[0, ST]] + list(bo_ap.ap)))
            pt_cm = tc.tile_pool(name="ptp", bufs=8)
            ptp = pt_cm.__enter__()
            psc_cm = tc.tile_pool(name="psc", bufs=2, space="PSUM")
            pscp = psc_cm.__enter__()
            po_cm = tc.tile_pool(name="po", bufs=2, space="PSUM")
            pop = po_cm.__enter__()
            psk_cm = tc.tile_pool(name="psk", bufs=2, space="PSUM")
            pskp = psk_cm.__enter__()

            def proj_chunks_for_pair(g):
                # closures emitting one tensor-engine chunk each
                wk_pr = stage_pair_weights(wk_d, g, "wkpr")
                wq_pr = stage_pair_weights(wq_d, g, "wqpr")
                chunks = []
                for sg in range(NSG):
                    chunks.append(
                        lambda sg=sg: k_proj_block(wk_pr, g, sg, pskp))
                for sg in range(NSG):
                    chunks.append(
                        lambda sg=sg: q_proj_block(wq_pr, g, sg, pskp))
                return chunks

            def wo_chunks(cs):
                chunks = []
                for c in cs:
                    def ch(c=c):
                        nc.gpsimd.dma_start(wob[:, c, :],
                                            wo_d.ap()[c * ST:(c + 1) * ST, :])
                    chunks.append(ch)
                return chunks

            # pair 0 was projected during the phase-A V pass

            def attention_pass(g, h2, p, pending_chunks, prev_fin):
                """Emit attention for (pair g, head h2, q-pass p).

                Slot j of the pass covers abs q-tile 2j+p (qt/cat columns
                are strided by 2 tiles).  pending_chunks: proj/wo closures
                drained ~evenly into the k-loop.  prev_fin: deferred
                normalization-finalize closures of the previous pass,
                drained after the first k-iterations.  Returns this pass's
                own finalize closures."""
                r = h2 * HD
                sl_k = slots_k[p]
                kf, kl = kfirst[p], klast[p]
                bsl, bkl, bfa = bank_slots[p], bklast[p], bank_fast[p]
                # strided views: column of slot j = (2j+p)*ST
                qtv = qt_t[g][r:r + HD, :].rearrange(
                    "e (j two c) -> e j two c", two=NP, c=ST)
                catv = cat[g][r:r + HD, :].rearrange(
                    "e (j two c) -> e j two c", two=NP, c=ST)
                po = {}
                for b_ in range(NB):
                    if bsl[b_]:
                        po[b_] = pop.tile([HD + 1, 512], F32, tag="po",
                                          name=f"po{g}_{h2}_{p}_{b_}")
                        if not bfa[b_]:
                            nc.vector.memset(po[b_][:, :], 0.0)

                active_ks = [k for k in range(NKT) if sl_k[k]]
                n_it = max(1, (len(active_ks) * 3) // 4)
                drain_every = max(1, n_it // (len(pending_chunks) + 1)) \
                    if pending_chunks else 0

                norm_state = {}

                def norm_pre(b_):
                    # PSUM-side reads: frees the po slot early; no tensor op
                    ltmp = p2s.tile([1, 512], F32, tag="ltmp", name="ltmp",
                                    bufs=1)
                    nc.vector.tensor_copy(ltmp[:, :], po[b_][HD:HD + 1, :])
                    rec = p2s.tile([1, 512], F32, tag="rec", name="rec",
                                   bufs=1)
                    nc.vector.reciprocal_approx_fast(rec[:, :], ltmp[:, :])
                    rec16 = p2s.tile([1, 512], BF16, tag="rec16", name="rec16",
                                     bufs=2)
                    nc.vector.tensor_copy(rec16[:, :], rec[:, :])
                    cslice = catv[:, 4 * b_:4 * b_ + 4, p, :]
                    nc.vector.tensor_copy(
                        cslice,
                        po[b_][0:HD, :].rearrange("e (j c) -> e j c", c=ST))
                    norm_state[b_] = (rec16, cslice)

                def norm_fin(b_):
                    rec16, cslice = norm_state[b_]
                    recps = pskp.tile([HD, 512], F32, tag="psk", name="recps")
                    nc.tensor.matmul(recps[:, :], ones1[:, :], rec16[:, :],
                                     start=True, stop=True)
                    nc.vector.tensor_mul(
                        cslice, cslice,
                        recps[:, :].rearrange("e (j c) -> e j c", c=ST))

                def emit_av(item, paired, pt):
                    done_banks = []
                    for par, (k, runs) in enumerate(item):
                        for run in runs:
                            sub = [run[0]]
                            subs = []
                            for j in run[1:]:
                                if kf[j] == kf[sub[0]]:
                                    sub.append(j)
                                else:
                                    subs.append(sub)
                                    sub = [j]
                            subs.append(sub)
                            for sub_ in subs:
                                ja, jb = sub_[0], sub_[-1]
                                b_ = ja // 4
                                fast = bfa[b_]
                                co = (par * 512 + (ja - 4 * b_) * ST
                                      if paired else ja * ST)
                                nc.tensor.matmul(
                                    po[b_][0:HD + 1,
                                           (ja - 4 * b_) * ST:
                                           (jb + 1 - 4 * b_) * ST],
                                    vb[:, k, 2 * g + h2, :],
                                    pt[:, co:co + (jb + 1 - ja) * ST],
                                    start=(fast and k == kf[ja]),
                                    stop=(fast and k == bkl[b_]),
                                    skip_group_check=not fast)
                                if fast and k == bkl[b_]:
                                    done_banks.append(b_)
                    return done_banks

                def runs_of(sl):
                    runs = []
                    run = [sl[0]]
                    for j in sl[1:]:
                        if j == run[-1] + 1 and j // 4 == run[0] // 4:
                            run.append(j)
                        else:
                            runs.append(run)
                            run = [j]
                    runs.append(run)
                    return runs

                def is_narrow(runs):
                    return (len(runs) == 1 and
                            (runs[0][-1] - runs[0][0] + 1) * ST <= 512)

                # batch: pair up narrow k-tiles (single run <= 512 wide) so
                # one exp ACTIVATE serves two k-tiles
                items = []
                i = 0
                while i < len(active_ks):
                    k = active_ks[i]
                    rk = runs_of(sl_k[k])
                    if is_narrow(rk) and i + 1 < len(active_ks):
                        k2 = active_ks[i + 1]
                        rk2 = runs_of(sl_k[k2])
                        if is_narrow(rk2):
                            items.append([(k, rk), (k2, rk2)])
                            i += 2
                            continue
                    items.append([(k, rk)])
                    i += 1

                def colof(j, par, paired):
                    # flat column of slot j within psc/pt for this sub-tile
                    if paired:
                        return par * 512 + (j - 4 * (j // 4)) * ST
                    return j * ST

                pending = []
                for ii, item in enumerate(items):
                    paired = len(item) == 2
                    psc = pscp.tile([ST, NJ * ST], F32, tag="psc", name="psc")
                    for par, (k, runs) in enumerate(item):
                        for run in runs:
                            ja, jb = run[0], run[-1]
                            w = (jb + 1 - ja) * ST
                            co = colof(ja, par, paired)
                            nc.tensor.matmul(
                                psc[:, co:co + w],
                                kt_t[g][r:r + HD, k * ST:(k + 1) * ST],
                                qtv[:, ja:jb + 1, p, :],
                                start=True, stop=True)
                    pt = ptp.tile([ST, NJ * ST], BF16, tag="pt", name="pt")
                    if paired:
                        # one exp over both sub-tiles via a strided 3-dim AP
                        o0 = min(colof(k_r[0][0], 0, True)
                                 for (kk, k_r) in item) % 512
                        o1 = max(colof(k_r[0][-1], 0, True) % 512 + ST
                                 for (kk, k_r) in item)
                        psc2 = psc[:, :].rearrange("p (a c) -> p a c", a=2)
                        pt2 = pt[:, :].rearrange("p (a c) -> p a c", a=2)
                        nc.scalar.activation(
                            pt2[:, :, o0:o1], psc2[:, :, o0:o1],
                            mybir.ActivationFunctionType.Exp,
                            scale=1.0 / float(np.sqrt(HD)))
                    else:
                        k, runs = item[0]
                        sl = sl_k[k]
                        jaT, jbT = sl[0], sl[-1]
                        nc.scalar.activation(
                            pt[:, jaT * ST:(jbT + 1) * ST],
                            psc[:, jaT * ST:(jbT + 1) * ST],
                            mybir.ActivationFunctionType.Exp,
                            scale=1.0 / float(np.sqrt(HD)))
                    for par, (k, runs) in enumerate(item):
                        for j in sl_k[k]:
                            if cls[p, j, k] == 2:
                                m = mixed_idx[(p, j, k)]
                                co = colof(j, par, paired)
                                nc.vector.tensor_mul(
                                    pt[:, co:co + ST],
                                    pt[:, co:co + ST],
                                    mtb[:, m, :])
                    pending.append((item, paired, pt))
                    if len(pending) > 1:
                        for b_ in emit_av(*pending.pop(0)):
                            norm_pre(b_)
                            if b_ != NB - 1:
                                # groups completing mid-pass finalize in-pass
                                norm_fin(b_)
                    if prev_fin and ii == 2:
                        while prev_fin:
                            prev_fin.pop(0)()
                    if pending_chunks and drain_every and \
                            ii % drain_every == drain_every - 1:
                        pending_chunks.pop(0)()
                for args in pending:
                    for b_ in emit_av(*args):
                        norm_pre(b_)
                        if b_ != NB - 1:
                            norm_fin(b_)
                while pending_chunks:
                    pending_chunks.pop(0)()
                # slow path for masks where a group never hits bklast (not
                # bank_fast): normalize any group not yet handled
                fins = []
                for b_ in range(NB):
                    if bsl[b_] and b_ not in norm_state:
                        norm_pre(b_)
                        if b_ != NB - 1:
                            norm_fin(b_)
                # the last group's tensor finalize is deferred into the
                # next pass's stream (returned to the caller)
                if bsl[NB - 1]:
                    fins.append(lambda: norm_fin(NB - 1))
                return fins

            fins = []
            for g in range(NG):
                chunks = proj_chunks_for_pair(g + 1) if g + 1 < NG else []
                if g == NG - 2:
                    chunks += wo_chunks(range(0, 4))
                if g == NG - 1:
                    chunks += wo_chunks(range(4, NDC))
                # split interleaved chunks between the 4 (head, pass) units
                nu = 4
                per = [chunks[(len(chunks) * u) // nu:
                              (len(chunks) * (u + 1)) // nu]
                       for u in range(nu)]
                for u, (h2, pq) in enumerate(
                        [(0, 0), (0, 1), (1, 0), (1, 1)]):
                    nf = attention_pass(g, h2, pq, per[u], fins)
                    fins = fins + nf
                # pair g's concat is complete once its deferred finalizes
                # run; exchange it with the batch peer, overlapped with
                # pair g+1's attention
                for f in fins:
                    f()
                fins = []
                nc.sync.dma_start(ag_in[g][:, :], cat[g][:, :])
                nc.gpsimd.collective_compute(
                    "AllGather",
                    mybir.AluOpType.bypass,
                    replica_groups=CC_GROUPS,
                    ins=[ag_in[g].opt()],
                    outs=[ag_out[g].opt()],
                )
                for rr in range(2):
                    nc.sync.dma_start(catx[g][:, rr, :],
                                      ag_out[g][rr, :, :])

            psk_cm.__exit__(None, None, None)
            po_cm.__exit__(None, None, None)
            psc_cm.__exit__(None, None, None)
            pt_cm.__exit__(None, None, None)

            # ---- phase D: fc_out from the gathered full concat ---------
            # contraction chunk order puts pair 3 (the last AllGather)
            # last in every tile's accumulation chain
            chunk_src = [(g, rr) for g in range(NG) for rr in range(2)]
            with (
                tc.tile_pool(name="p3s", bufs=6) as p3s,
                tc.tile_pool(name="psy", bufs=8, space="PSUM") as psy,
            ):
                for ta in range(NKT):
                    py = [psy.tile([ST, 512], F32, tag="py",
                                   name=f"py{ta}_{n}") for n in range(2)]
                    for ci, (g, rr) in enumerate(chunk_src):
                        c = rr * NG + g  # global Wo contraction chunk
                        for n in range(2):
                            nc.tensor.matmul(
                                py[n][:, :],
                                catx[g][:, rr, ta * ST:(ta + 1) * ST],
                                wob[:, c, n * 512:(n + 1) * 512],
                                start=(ci == 0), stop=(ci == NDC - 1))
                    ysb = p3s.tile([ST, D], F32, tag="ysb", name="ysb")
                    for n in range(2):
                        nc.vector.tensor_add(
                            ysb[:, n * 512:(n + 1) * 512], py[n][:, :],
                            bob[:, n * 512:(n + 1) * 512])
                    qeng = nc.sync if ta % 2 == 0 else nc.scalar
                    qeng.dma_start(
                        out_d.ap()[ta * ST:(ta + 1) * ST, :], ysb[:, :])

            wop_cm.__exit__(None, None, None)
            p2s_cm.__exit__(None, None, None)
            wp_cm.__exit__(None, None, None)
            xtp_cm.__exit__(None, None, None)

    nc.compile()
    return nc


_CACHE = {}
LAST_RESULT = None


def _get_program(mask):
    key = mask.tobytes()
    if key not in _CACHE:
        cls, mixed, midx, n_maskt = _classify(mask)
        _CACHE[key] = (_build(cls, mixed, midx, n_maskt), cls, mixed, midx,
                       n_maskt)
    return _CACHE[key]


def kernel(x, mask, Wq, bq, Wk, bk, Wv, bv, Wo, bo):
    x = np.asarray(x, dtype=np.float32)
    mask = np.asarray(mask)
    nc, cls, mixed, midx, n_maskt = _get_program(mask)

    mt = np.zeros((n_maskt, ST, ST), dtype=ml_dtypes.bfloat16)
    for (p, j, k) in mixed:
        t = 2 * j + p
        blk = mask[t * ST:(t + 1) * ST, k * ST:(k + 1) * ST]
        mt[midx[(p, j, k)]] = (blk != 0).T.astype(ml_dtypes.bfloat16)

    Wq = np.asarray(Wq, dtype=np.float32).astype(ml_dtypes.bfloat16)
    Wk = np.asarray(Wk, dtype=np.float32).astype(ml_dtypes.bfloat16)
    Wv = np.asarray(Wv, dtype=np.float32).astype(ml_dtypes.bfloat16)
    Wo = np.ascontiguousarray(
        np.asarray(Wo, dtype=np.float32).astype(ml_dtypes.bfloat16))
    bq = np.asarray(bq, dtype=np.float32)
    bk = np.asarray(bk, dtype=np.float32)
    bv = np.asarray(bv, dtype=np.float32)
    bo = np.ascontiguousarray(np.asarray(bo, dtype=np.float32))

    x_bf = x.astype(ml_dtypes.bfloat16)
    in_maps = []
    for c in range(N_CORES):
        b, hh = c // 2, c % 2
        hs = slice(hh * HL, (hh + 1) * HL)
        m = {
            "x": np.ascontiguousarray(x_bf[b]),
            "wq": np.ascontiguousarray(Wq[hs]),
            "wk": np.ascontiguousarray(Wk[hs]),
            "wv": np.ascontiguousarray(Wv[hs]),
            "wo": Wo,
            "bq": np.ascontiguousarray(bq[hs]),
            "bk": np.ascontiguousarray(bk[hs]),
            "bv": np.ascontiguousarray(bv[hs]),
            "bo": bo,
            "maskt": mt,
        }
        in_maps.append(m)

    res = run_bass_kernel_spmd(
        nc, in_maps, core_ids=list(range(N_CORES)),
        trace=os.environ.get("BASS_KERNEL_TRACE", "0") == "1")
    global LAST_RESULT
    LAST_RESULT = res

    out = np.empty((B, S, D), dtype=np.float32)
    for c in range(N_CORES):
        b, hh = c // 2, c % 2
        sl = slice(hh * NJ * ST, (hh + 1) * NJ * ST)
        out[b, sl, :] = res.results[c]["out"][sl]
    return out


# revision 24
# speedup vs baseline: 1.1027x; 1.1027x over previous
"""Trainium2 Bass kernel: causal multi-head attention (B=4,S=2048,D=1024,H=16).

Sharding (8 cores, pair-wise AllGather): core c -> batch b=c//2,
head-half hh=c%2 (heads hh*8..hh*8+7).  Each core projects K/V/Q for its
8 heads over the FULL sequence (no duplicated K/V work), runs causal
attention for all 2048 queries (two interleaved q-passes of 8 x 128-row
slots per head), exchanges each completed head-pair's concat with its
batch peer via a pair-wise AllGather overlapped with the remaining
attention, then computes fc_out for all 2048 rows from the gathered
full concat; the host keeps each core's contiguous sequence half.

Device pipeline per core (all matmuls bf16, f32 accumulation), organized
to keep the tensor engine continuously busy:

  A: x^T via PE transposes (casts/evacuations alternate ScalarE/VectorE,
     DMAs spread over the sync+scalar queues), all weight tensors loaded
     via casting gpsimd DMAs (f32->bf16 in the DMA), then a dense
     V-projection pass over all 16 s-tiles with pair-0 K^T/Q^T blocks
     interleaved.
  C: per (head, q-pass), per k-tile: scores^T into a 2-bank PSUM tile
     (double buffered), ONE exp ACTIVATE per k-tile (narrow adjacent
     k-tiles are paired into a single strided ACTIVATE), 0/1 mask
     multiply on mixed tiles only, out^T accumulation per 512-col group
     with ones-augmented V (row 64 = softmax denominator).  Softmax
     normalization is split: the PSUM-side prep is emitted as soon as a
     group's accumulation completes, while the tensor-engine finalize
     for the last group is deferred into the next (head, pass) stream.
     K^T/Q^T projection chunks for pair g+1 (and the Wo casting DMAs)
     are interleaved into pair g's attention stream.
  D: as soon as pair g's attention (4 head/pass units) completes, its
     concat tile cat[g] (bf16, 512KB) is DMAd to DRAM and a pair-wise
     AllGather fires, hidden under the remaining attention work (only
     pair 3's gather is partially exposed, covered by ordering its
     fc_out chunks last).  fc_out reads ONLY the gathered concat (both
     ranks' heads, global order) and computes all 16 q-tiles with the
     full Wo; the host keeps each core's sequence half.

The program is specialized at build time to the mask's block structure
per q-pass (skip all-zero blocks / skip masking on all-ones blocks);
this is computed from the actual mask input, so it stays correct for
any mask.
"""

import os
import numpy as np
import ml_dtypes

import concourse.bass as bass
import concourse.mybir as mybir
import concourse.tile as tile
from concourse import bacc
from concourse.bass_utils import run_bass_kernel_spmd

B, S, D, H, HD = 4, 2048, 1024, 16, 64
N_CORES = 8
ST = 128               # tile edge (partition size)
NKT = S // ST          # 16 key tiles
NJ = 8                 # q slots per pass (8*128 = 1024 rows)
NP = 2                 # q passes per head (slot j of pass p = abs tile 2j+p)
HL = H // 2            # 8 local heads per core
NDC = D // ST          # 8 contraction chunks
NG = HL // 2           # 4 local head pairs (2 heads per 128 partitions)
NSG = S // 512         # 4 s-groups of 512 cols for K^T/Q^T projections
NB = NJ // 4           # 2 x 512-col output groups of slots per pass
NWOC = (HL * HD) // ST  # 4 local Wo contraction chunks

F32 = mybir.dt.float32
BF16 = mybir.dt.bfloat16

# batch peers are paired as (c, c+2) so the two cores of a replica pair
# do not share an HBM port (adjacent cores do under LNC1)
CC_GROUPS = [[0, 2], [1, 3], [4, 6], [5, 7]]
CORE_BH = {0: (0, 0), 2: (0, 1), 1: (1, 0), 3: (1, 1),
           4: (2, 0), 6: (2, 1), 5: (3, 0), 7: (3, 1)}


def _classify(mask: np.ndarray):
    """Block structure of the mask per q-pass.

    Returns (cls[NP][NJ][NKT] in {0 skip,1 full,2 mixed}, mixed list of
    (p,j,k), mixed->dedup-index map, number of distinct mask tiles).
    """
    cls = np.zeros((NP, NJ, NKT), dtype=int)
    for p in range(NP):
        for j in range(NJ):
            t = 2 * j + p
            for k in range(NKT):
                blk = mask[t * ST:(t + 1) * ST, k * ST:(k + 1) * ST]
                if (blk != 0).all():
                    cls[p, j, k] = 1
                elif (blk == 0).all():
                    cls[p, j, k] = 0
                else:
                    cls[p, j, k] = 2
            # close interior holes so the computed k-range is contiguous
            nz = np.nonzero(cls[p, j])[0]
            if len(nz):
                for k in range(nz[0], nz[-1] + 1):
                    if cls[p, j, k] == 0:
                        cls[p, j, k] = 2
    mixed = [(p, j, k) for p in range(NP) for j in range(NJ)
             for k in range(NKT) if cls[p, j, k] == 2]
    dedup = {}
    midx = {}
    for (p, j, k) in mixed:
        t = 2 * j + p
        key = mask[t * ST:(t + 1) * ST, k * ST:(k + 1) * ST].tobytes()
        if key not in dedup:
            dedup[key] = len(dedup)
        midx[(p, j, k)] = dedup[key]
    return cls, mixed, midx, max(len(dedup), 1)


def _build(cls, mixed, mixed_idx, n_maskt):
    """Build the (uniform, SPMD) Bass program for one core's shard."""
    nc = bacc.Bacc("TRN2", target_bir_lowering=False, debug=False,
                   num_devices=N_CORES)

    x_d = nc.dram_tensor("x", [S, D], BF16, kind="ExternalInput")
    wq_d = nc.dram_tensor("wq", [HL, D, HD], BF16, kind="ExternalInput")
    wk_d = nc.dram_tensor("wk", [HL, D, HD], BF16, kind="ExternalInput")
    wv_d = nc.dram_tensor("wv", [HL, D, HD], BF16, kind="ExternalInput")
    wo_d = nc.dram_tensor("wo", [D, D], BF16, kind="ExternalInput")
    bq_d = nc.dram_tensor("bq", [HL, HD], F32, kind="ExternalInput")
    bk_d = nc.dram_tensor("bk", [HL, HD], F32, kind="ExternalInput")
    bv_d = nc.dram_tensor("bv", [HL, HD], F32, kind="ExternalInput")
    bo_d = nc.dram_tensor("bo", [D], F32, kind="ExternalInput")
    mt_d = nc.dram_tensor("maskt", [n_maskt, ST, ST], BF16, kind="ExternalInput")
    out_d = nc.dram_tensor("out", [S, D], F32, kind="ExternalOutput")

    # per-pass mask/block structure
    slots_k = [[[j for j in range(NJ) if cls[p, j, k]] for k in range(NKT)]
               for p in range(NP)]
    kfirst = [{} for _ in range(NP)]
    klast = [{} for _ in range(NP)]
    for p in range(NP):
        for j in range(NJ):
            ks = [k for k in range(NKT) if cls[p, j, k]]
            if ks:
                kfirst[p][j], klast[p][j] = ks[0], ks[-1]
    bank_slots = [[[j for j in range(4 * b_, 4 * b_ + 4) if j in kfirst[p]]
                   for b_ in range(NB)] for p in range(NP)]
    bklast = [{b_: max(klast[p][j] for j in bank_slots[p][b_])
               for b_ in range(NB) if bank_slots[p][b_]} for p in range(NP)]
    bank_fast = [{b_: len({kfirst[p][j] for j in bank_slots[p][b_]}) == 1
                  for b_ in range(NB) if bank_slots[p][b_]}
                 for p in range(NP)]

    from concourse.masks import make_identity

    with tile.TileContext(nc) as tc:
        with (
            tc.tile_pool(name="persist", bufs=1) as pp,
            tc.tile_pool(name="dram", bufs=1, space="DRAM") as dram,
        ):
            # ---- persistent SBUF tensors -------------------------------
            kt_t = [pp.tile([ST, S], BF16, name=f"ktg{g}", tag=f"ktg{g}")
                    for g in range(NG)]
            qt_t = [pp.tile([ST, S], BF16, name=f"qtg{g}", tag=f"qtg{g}")
                    for g in range(NG)]
            vb = pp.tile([ST, NKT, HL, HD + 1], BF16, name="vb", tag="vb")
            cat = [pp.tile([ST, S], BF16, name=f"catg{g}", tag=f"catg{g}")
                   for g in range(NG)]
            # gathered concat: catx[g][:, r, :] = rank r's pair g
            catx = [pp.tile([ST, 2, S], BF16, name=f"catx{g}",
                            tag=f"catx{g}") for g in range(NG)]
            ident = pp.tile([ST, ST], BF16, name="ident", tag="ident")
            ones1 = pp.tile([1, HD], BF16, name="ones1", tag="ones1")
            mtb = pp.tile([ST, max(n_maskt, 1), ST], BF16, name="mtb",
                          tag="mtb")

            # AllGather bounce buffers (one per local head pair)
            ag_in = [dram.tile([ST, S], BF16, name=f"agin{g}",
                               tag=f"agin{g}") for g in range(NG)]
            ag_out = [dram.tile([2, ST, S], BF16, name=f"agout{g}",
                                tag=f"agout{g}") for g in range(NG)]

            nc.vector.memset(vb[:, :, :, HD:HD + 1], 1.0)
            nc.vector.memset(ones1[:, :], 1.0)
            junk = pp.tile([1, 1], F32, name="junk", tag="junk")
            nc.vector.memset(junk[:, :], 0.0)
            make_identity(nc, ident[:, :])
            nc.scalar.dma_start(mtb[:, :, :],
                                mt_d.ap().rearrange("m p f -> p m f"))

            def load_bias_pair(pool, bias_d, name):
                # [128, NG] f32: partition = (h%2)*64+e, column = pair idx
                t = pool.tile([ST, NG], F32, name=name, tag=name, bufs=1)
                src = bias_d.ap()
                nc.scalar.dma_start(
                    t[:, :],
                    bass.AP(tensor=src.tensor, offset=src.offset,
                            ap=[[1, ST], [ST, NG]]))
                return t

            # x^T tiles (live until the last K/Q projection)
            xtp_cm = tc.tile_pool(name="xtp", bufs=1, side="right")
            xtp = xtp_cm.__enter__()
            xt_all = xtp.tile([ST, NDC, NKT, ST], BF16, name="xt_all",
                              tag="xt_all")

            # weight-pair destination pool (lives phase A .. attention)
            wp_cm = tc.tile_pool(name="wpair", bufs=2)
            wp = wp_cm.__enter__()

            bkp = load_bias_pair(pp, bk_d, "bkp")
            bqp = load_bias_pair(pp, bq_d, "bqp")

            def k_proj_block(wpr, g, sg, pool, pbufs=2):
                psk = pool.tile([ST, 512], F32, tag="psk", name="psk",
                                bufs=pbufs)
                for c in range(NDC):
                    nc.tensor.matmul(
                        psk[:, :], wpr[:, c, :],
                        xt_all[:, c, 4 * sg:4 * (sg + 1), :],
                        start=(c == 0), stop=(c == NDC - 1))
                nc.vector.tensor_scalar(
                    kt_t[g][:, sg * 512:(sg + 1) * 512],
                    psk[:, :], bkp[:, g:g + 1], None,
                    mybir.AluOpType.add)

            def q_proj_block(wpr, g, sg, pool, pbufs=2):
                psk = pool.tile([ST, 512], F32, tag="psk", name="psk",
                                bufs=pbufs)
                for c in range(NDC):
                    nc.tensor.matmul(
                        psk[:, :], wpr[:, c, :],
                        xt_all[:, c, 4 * sg:4 * (sg + 1), :],
                        start=(c == 0), stop=(c == NDC - 1))
                nc.vector.tensor_scalar(
                    qt_t[g][:, sg * 512:(sg + 1) * 512],
                    psk[:, :], bqp[:, g:g + 1], None,
                    mybir.AluOpType.add)

            def stage_pair_weights(w_d, g, tag):
                # casting gpsimd DMAs straight into the [128, NDC, 128]
                # stationary-pair layout (f32 -> bf16 in the DMA)
                wpr = wp.tile([ST, NDC, ST], BF16, name=f"{tag}{g}", tag=tag)
                for h2 in range(2):
                    src = w_d.ap()[2 * g + h2].rearrange(
                        "(c p) e -> p c e", p=ST)
                    nc.gpsimd.dma_start(
                        wpr[:, :, h2 * HD:(h2 + 1) * HD], src)
                return wpr

            # ---- phase A: x^T, then a dense V pass ---------------------
            with (
                tc.tile_pool(name="p1a", bufs=2) as p1a,
                tc.tile_pool(name="pv", bufs=1) as pv,
                tc.tile_pool(name="ppst", bufs=4, space="PSUM") as ppst,
                tc.tile_pool(name="ppsv", bufs=3, space="PSUM") as ppsv,
            ):
                wvb = pv.tile([ST, NDC, HL, HD], BF16, name="wvb", tag="wvb",
                              bufs=1)
                bvf = pv.tile([ST, HL, HD], F32, name="bvf", tag="bvf",
                              bufs=1)
                # gpsimd casting-DMA order matches consumption order:
                # all 8 V heads (dense pass), then pair-0 K/Q weights
                for hh in range(HL):
                    srcw = wv_d.ap()[hh].rearrange("(c p) e -> p c e", p=ST)
                    nc.gpsimd.dma_start(wvb[:, :, hh, :], srcw)
                _stage0 = (stage_pair_weights(wk_d, 0, "wkpr"),
                           stage_pair_weights(wq_d, 0, "wqpr"))

                evac_i = 0

                def transpose_tile(dst_all, sti, xsrc_ap):
                    nonlocal evac_i
                    xb = p1a.tile([ST, D], BF16, tag="xb", name="xb",
                                  bufs=6)
                    qeng = nc.sync if (evac_i // 8) % 2 == 0 else nc.scalar
                    qeng.dma_start(xb[:, :], xsrc_ap)
                    for c4 in range(NDC // 4):
                        pst4 = ppst.tile([ST, 4, ST], BF16, tag="pst",
                                         name="pst")
                        for i in range(4):
                            c = 4 * c4 + i
                            nc.tensor.transpose(
                                pst4[:, i, :], xb[:, c * ST:(c + 1) * ST],
                                ident[:, :])
                        if evac_i % 2 == 1:
                            nc.scalar.copy(
                                dst_all[:, 4 * c4:4 * c4 + 4, sti, :],
                                pst4[:, :, :])
                        else:
                            nc.vector.tensor_copy(
                                dst_all[:, 4 * c4:4 * c4 + 4, sti, :],
                                pst4[:, :, :])
                        evac_i += 1

                # preload the Exp activation table early
                nc.scalar.activation(junk[:, :], junk[:, :],
                                     mybir.ActivationFunctionType.Exp)
                # pair-0 K/Q chunks interleaved as their xt tiles land
                _p0 = []
                for sg in range(NSG):
                    _p0.append((k_proj_block, _stage0[0], sg, ppst))
                    _p0.append((q_proj_block, _stage0[1], sg, ppst))

                def vproj(st):
                    # dense V projection for s-tile st (all 8 heads)
                    psv = ppsv.tile([ST, HL * HD], F32, tag="psv",
                                    name="psv")
                    for c in range(NDC):
                        nc.tensor.matmul(
                            psv[:, :],
                            xt_all[:, c, st, :],
                            wvb[:, c, :, :],
                            start=(c == 0), stop=(c == NDC - 1))
                    nc.vector.tensor_add(
                        vb[:, st, :, 0:HD],
                        psv[:, :].rearrange("p (h e) -> p h e", h=HL),
                        bvf[:, :, :])
                    # K/Q s-group j//4 only needs xt tiles <= st
                    if st % 4 == 3:
                        for _ in range(2):
                            fn, w_, sg_, pl_ = _p0.pop(0)
                            fn(w_, 0, sg_, pl_, 1)

                # V-projection (and pair-0 K/Q) interleave into the
                # transpose stream so the PE has work while x loads
                for st in range(NKT):
                    transpose_tile(xt_all, st,
                                   x_d.ap()[st * ST:(st + 1) * ST, :])
                    if st == 0:
                        srcv = bv_d.ap()
                        nc.scalar.dma_start(
                            bvf[:, :, :],
                            bass.AP(tensor=srcv.tensor, offset=srcv.offset,
                                    ap=[[0, ST]] + list(srcv.ap)))
                    if st >= 1:
                        vproj(st - 1)
                vproj(NKT - 1)
                while _p0:
                    fn, w_, sg_, pl_ = _p0.pop(0)
                    fn(w_, 0, sg_, pl_, 1)

            # ---- phases B/C/D: projections + attention + fc_out --------
            # PSUM budget: psc 2x2 banks + po 2x1 banks + psk 2x1 = 8.
            p2s_cm = tc.tile_pool(name="p2s", bufs=2)
            p2s = p2s_cm.__enter__()
            wob = pp.tile([ST, NDC, D], BF16, name="wob", tag="wob")
            bob = pp.tile([ST, D], BF16, name="bob", tag="bob")
            bo_ap = bo_d.ap()
            nc.gpsimd.dma_start(
                bob[:, :],
                bass.AP(tensor=bo_ap.tensor, offset=bo_ap.offset,
                        ap=[[0, ST]] + list(bo_ap.ap)))
            pt_cm = tc.tile_pool(name="ptp", bufs=8)
            ptp = pt_cm.__enter__()
            psc_cm = tc.tile_pool(name="psc", bufs=2, space="PSUM")
            pscp = psc_cm.__enter__()
            po_cm = tc.tile_pool(name="po", bufs=2, space="PSUM")
            pop = po_cm.__enter__()
            psk_cm = tc.tile_pool(name="psk", bufs=2, space="PSUM")
            pskp = psk_cm.__enter__()

            def proj_chunks_for_pair(g):
                # closures emitting one tensor-engine chunk each
                wk_pr = stage_pair_weights(wk_d, g, "wkpr")
                wq_pr = stage_pair_weights(wq_d, g, "wqpr")
                chunks = []
                for sg in range(NSG):
                    chunks.append(
                        lambda sg=sg: k_proj_block(wk_pr, g, sg, pskp))
                for sg in range(NSG):
                    chunks.append(
                        lambda sg=sg: q_proj_block(wq_pr, g, sg, pskp))
                return chunks

            def wo_chunks(cs):
                chunks = []
                for c in cs:
                    def ch(c=c):
                        nc.gpsimd.dma_start(wob[:, c, :],
                                            wo_d.ap()[c * ST:(c + 1) * ST, :])
                    chunks.append(ch)
                return chunks

            # pair 0 was projected during the phase-A V pass

            def attention_pass(g, h2, p, pending_chunks, prev_fin):
                """Emit attention for (pair g, head h2, q-pass p).

                Slot j of the pass covers abs q-tile 2j+p (qt/cat columns
                are strided by 2 tiles).  pending_chunks: proj/wo closures
                drained ~evenly into the k-loop.  prev_fin: deferred
                normalization-finalize closures of the previous pass,
                drained after the first k-iterations.  Returns this pass's
                own finalize closures."""
                r = h2 * HD
                sl_k = slots_k[p]
                kf, kl = kfirst[p], klast[p]
                bsl, bkl, bfa = bank_slots[p], bklast[p], bank_fast[p]
                # strided views: column of slot j = (2j+p)*ST
                qtv = qt_t[g][r:r + HD, :].rearrange(
                    "e (j two c) -> e j two c", two=NP, c=ST)
                catv = cat[g][r:r + HD, :].rearrange(
                    "e (j two c) -> e j two c", two=NP, c=ST)
                po = {}
                for b_ in range(NB):
                    if bsl[b_]:
                        po[b_] = pop.tile([HD + 1, 512], F32, tag="po",
                                          name=f"po{g}_{h2}_{p}_{b_}")
                        if not bfa[b_]:
                            nc.vector.memset(po[b_][:, :], 0.0)

                active_ks = [k for k in range(NKT) if sl_k[k]]
                n_it = max(1, (len(active_ks) * 3) // 4)
                drain_every = max(1, n_it // (len(pending_chunks) + 1)) \
                    if pending_chunks else 0

                norm_state = {}

                def norm_pre(b_):
                    # PSUM-side reads: frees the po slot early; no tensor op
                    ltmp = p2s.tile([1, 512], F32, tag="ltmp", name="ltmp",
                                    bufs=1)
                    nc.vector.tensor_copy(ltmp[:, :], po[b_][HD:HD + 1, :])
                    rec = p2s.tile([1, 512], F32, tag="rec", name="rec",
                                   bufs=1)
                    nc.vector.reciprocal_approx_fast(rec[:, :], ltmp[:, :])
                    rec16 = p2s.tile([1, 512], BF16, tag="rec16", name="rec16",
                                     bufs=2)
                    nc.vector.tensor_copy(rec16[:, :], rec[:, :])
                    cslice = catv[:, 4 * b_:4 * b_ + 4, p, :]
                    nc.vector.tensor_copy(
                        cslice,
                        po[b_][0:HD, :].rearrange("e (j c) -> e j c", c=ST))
                    norm_state[b_] = (rec16, cslice)

                def norm_fin(b_):
                    rec16, cslice = norm_state[b_]
                    recps = pskp.tile([HD, 512], F32, tag="psk", name="recps")
                    nc.tensor.matmul(recps[:, :], ones1[:, :], rec16[:, :],
                                     start=True, stop=True)
                    nc.vector.tensor_mul(
                        cslice, cslice,
                        recps[:, :].rearrange("e (j c) -> e j c", c=ST))

                def emit_av(item, paired, pt):
                    done_banks = []
                    for par, (k, runs) in enumerate(item):
                        for run in runs:
                            sub = [run[0]]
                            subs = []
                            for j in run[1:]:
                                if kf[j] == kf[sub[0]]:
                                    sub.append(j)
                                else:
                                    subs.append(sub)
                                    sub = [j]
                            subs.append(sub)
                            for sub_ in subs:
                                ja, jb = sub_[0], sub_[-1]
                                b_ = ja // 4
                                fast = bfa[b_]
                                co = (par * 512 + (ja - 4 * b_) * ST
                                      if paired else ja * ST)
                                nc.tensor.matmul(
                                    po[b_][0:HD + 1,
                                           (ja - 4 * b_) * ST:
                                           (jb + 1 - 4 * b_) * ST],
                                    vb[:, k, 2 * g + h2, :],
                                    pt[:, co:co + (jb + 1 - ja) * ST],
                                    start=(fast and k == kf[ja]),
                                    stop=(fast and k == bkl[b_]),
                                    skip_group_check=not fast)
                                if fast and k == bkl[b_]:
                                    done_banks.append(b_)
                    return done_banks

                def runs_of(sl):
                    runs = []
                    run = [sl[0]]
                    for j in sl[1:]:
                        if j == run[-1] + 1 and j // 4 == run[0] // 4:
                            run.append(j)
                        else:
                            runs.append(run)
                            run = [j]
                    runs.append(run)
                    return runs

                def is_narrow(runs):
                    return (len(runs) == 1 and
                            (runs[0][-1] - runs[0][0] + 1) * ST <= 512)

                # batch: pair up narrow k-tiles (single run <= 512 wide) so
                # one exp ACTIVATE serves two k-tiles
                items = []
                i = 0
                while i < len(active_ks):
                    k = active_ks[i]
                    rk = runs_of(sl_k[k])
                    if is_narrow(rk) and i + 1 < len(active_ks):
                        k2 = active_ks[i + 1]
                        rk2 = runs_of(sl_k[k2])
                        if is_narrow(rk2):
                            items.append([(k, rk), (k2, rk2)])
                            i += 2
                            continue
                    items.append([(k, rk)])
                    i += 1

                def colof(j, par, paired):
                    # flat column of slot j within psc/pt for this sub-tile
                    if paired:
                        return par * 512 + (j - 4 * (j // 4)) * ST
                    return j * ST

                pending = []
                for ii, item in enumerate(items):
                    paired = len(item) == 2
                    psc = pscp.tile([ST, NJ * ST], F32, tag="psc", name="psc")
                    for par, (k, runs) in enumerate(item):
                        for run in runs:
                            ja, jb = run[0], run[-1]
                            w = (jb + 1 - ja) * ST
                            co = colof(ja, par, paired)
                            nc.tensor.matmul(
                                psc[:, co:co + w],
                                kt_t[g][r:r + HD, k * ST:(k + 1) * ST],
                                qtv[:, ja:jb + 1, p, :],
                                start=True, stop=True)
                    pt = ptp.tile([ST, NJ * ST], BF16, tag="pt", name="pt")
                    if paired:
                        # one exp over both sub-tiles via a strided 3-dim AP
                        o0 = min(colof(k_r[0][0], 0, True)
                                 for (kk, k_r) in item) % 512
                        o1 = max(colof(k_r[0][-1], 0, True) % 512 + ST
                                 for (kk, k_r) in item)
                        psc2 = psc[:, :].rearrange("p (a c) -> p a c", a=2)
                        pt2 = pt[:, :].rearrange("p (a c) -> p a c", a=2)
                        nc.scalar.activation(
                            pt2[:, :, o0:o1], psc2[:, :, o0:o1],
                            mybir.ActivationFunctionType.Exp,
                            scale=1.0 / float(np.sqrt(HD)))
                    else:
                        k, runs = item[0]
                        sl = sl_k[k]
                        jaT, jbT = sl[0], sl[-1]
                        nc.scalar.activation(
                            pt[:, jaT * ST:(jbT + 1) * ST],
                            psc[:, jaT * ST:(jbT + 1) * ST],
                            mybir.ActivationFunctionType.Exp,
                            scale=1.0 / float(np.sqrt(HD)))
                    for par, (k, runs) in enumerate(item):
                        for j in sl_k[k]:
                            if cls[p, j, k] == 2:
                                m = mixed_idx[(p, j, k)]
                                co = colof(j, par, paired)
                                nc.vector.tensor_mul(
                                    pt[:, co:co + ST],
                                    pt[:, co:co + ST],
                                    mtb[:, m, :])
                    pending.append((item, paired, pt))
                    if len(pending) > 1:
                        for b_ in emit_av(*pending.pop(0)):
                            norm_pre(b_)
                            if b_ != NB - 1:
                                # groups completing mid-pass finalize in-pass
                                norm_fin(b_)
                    if prev_fin and ii == 2:
                        while prev_fin:
                            prev_fin.pop(0)()
                    if pending_chunks and drain_every and \
                            ii % drain_every == drain_every - 1:
                        pending_chunks.pop(0)()
                for args in pending:
                    for b_ in emit_av(*args):
                        norm_pre(b_)
                        if b_ != NB - 1:
                            norm_fin(b_)
                while pending_chunks:
                    pending_chunks.pop(0)()
                # slow path for masks where a group never hits bklast (not
                # bank_fast): normalize any group not yet handled
                fins = []
                for b_ in range(NB):
                    if bsl[b_] and b_ not in norm_state:
                        norm_pre(b_)
                        if b_ != NB - 1:
                            norm_fin(b_)
                # the last group's tensor finalize is deferred into the
                # next pass's stream (returned to the caller)
                if bsl[NB - 1]:
                    fins.append(lambda: norm_fin(NB - 1))
                return fins

            fins = []
            for g in range(NG):
                chunks = proj_chunks_for_pair(g + 1) if g + 1 < NG else []
                if g == NG - 2:
                    chunks += wo_chunks(range(0, 4))
                if g == NG - 1:
                    chunks += wo_chunks(range(4, NDC))
                # split interleaved chunks between the 4 (head, pass) units
                nu = 4
                per = [chunks[(len(chunks) * u) // nu:
                              (len(chunks) * (u + 1)) // nu]
                       for u in range(nu)]
                for u, (h2, pq) in enumerate(
                        [(0, 0), (0, 1), (1, 0), (1, 1)]):
                    nf = attention_pass(g, h2, pq, per[u], fins)
                    fins = fins + nf
                # pair g's concat is complete once its deferred finalizes
                # run; exchange it with the batch peer, overlapped with
                # pair g+1's attention
                for f in fins:
                    f()
                fins = []
                nc.sync.dma_start(ag_in[g][:, :], cat[g][:, :])
                nc.gpsimd.collective_compute(
                    "AllGather",
                    mybir.AluOpType.bypass,
                    replica_groups=CC_GROUPS,
                    ins=[ag_in[g].opt()],
                    outs=[ag_out[g].opt()],
                )
                for rr in range(2):
                    nc.sync.dma_start(catx[g][:, rr, :],
                                      ag_out[g][rr, :, :])

            psk_cm.__exit__(None, None, None)
            po_cm.__exit__(None, None, None)
            psc_cm.__exit__(None, None, None)
            pt_cm.__exit__(None, None, None)
            p2s_cm.__exit__(None, None, None)
            wp_cm.__exit__(None, None, None)
            xtp_cm.__exit__(None, None, None)

            # ---- phase D: fc_out from the gathered full concat ---------
            # split around pair 3 (the last AllGather): a pre-pass over the
            # 6 early chunks runs for all 16 tiles (staged to bf16 SBUF,
            # bias included), covering the final gather's latency; a short
            # post-pass adds pair 3's two chunks and writes out.
            chunk_pre = [(g, rr) for g in range(NG - 1) for rr in range(2)]
            with (
                tc.tile_pool(name="p3s", bufs=16) as p3s,
                tc.tile_pool(name="p3f", bufs=4) as p3f,
                tc.tile_pool(name="psy", bufs=8, space="PSUM") as psy,
            ):
                pre = {}
                for ta in range(NKT):
                    py = [psy.tile([ST, 512], F32, tag="py",
                                   name=f"py{ta}_{n}") for n in range(2)]
                    for ci, (g, rr) in enumerate(chunk_pre):
                        c = rr * NG + g  # global Wo contraction chunk
                        for n in range(2):
                            nc.tensor.matmul(
                                py[n][:, :],
                                catx[g][:, rr, ta * ST:(ta + 1) * ST],
                                wob[:, c, n * 512:(n + 1) * 512],
                                start=(ci == 0),
                                stop=(ci == len(chunk_pre) - 1))
                    y16 = p3s.tile([ST, D], BF16, tag="y16", name="y16")
                    for n in range(2):
                        nc.vector.tensor_add(
                            y16[:, n * 512:(n + 1) * 512], py[n][:, :],
                            bob[:, n * 512:(n + 1) * 512])
                    pre[ta] = y16
                for ta in range(NKT):
                    py = [psy.tile([ST, 512], F32, tag="py",
                                   name=f"pz{ta}_{n}") for n in range(2)]
                    for ci, rr in enumerate(range(2)):
                        c = rr * NG + (NG - 1)
                        for n in range(2):
                            nc.tensor.matmul(
                                py[n][:, :],
                                catx[NG - 1][:, rr, ta * ST:(ta + 1) * ST],
                                wob[:, c, n * 512:(n + 1) * 512],
                                start=(ci == 0), stop=(ci == 1))
                    ysb = p3f.tile([ST, D], F32, tag="ysb", name="ysb")
                    for n in range(2):
                        nc.vector.tensor_add(
                            ysb[:, n * 512:(n + 1) * 512], py[n][:, :],
                            pre[ta][:, n * 512:(n + 1) * 512])
                    qeng = nc.sync if ta % 2 == 0 else nc.scalar
                    qeng.dma_start(
                        out_d.ap()[ta * ST:(ta + 1) * ST, :], ysb[:, :])

    nc.compile()
    return nc


_CACHE = {}
LAST_RESULT = None


def _get_program(mask):
    key = mask.tobytes()
    if key not in _CACHE:
        cls, mixed, midx, n_maskt = _classify(mask)
        _CACHE[key] = (_build(cls, mixed, midx, n_maskt), cls, mixed, midx,
                       n_maskt)
    return _CACHE[key]


def kernel(x, mask, Wq, bq, Wk, bk, Wv, bv, Wo, bo):
    x = np.asarray(x, dtype=np.float32)
    mask = np.asarray(mask)
    nc, cls, mixed, midx, n_maskt = _get_program(mask)

    mt = np.zeros((n_maskt, ST, ST), dtype=ml_dtypes.bfloat16)
    for (p, j, k) in mixed:
        t = 2 * j + p
        blk = mask[t * ST:(t + 1) * ST, k * ST:(k + 1) * ST]
        mt[midx[(p, j, k)]] = (blk != 0).T.astype(ml_dtypes.bfloat16)

    Wq = np.asarray(Wq, dtype=np.float32).astype(ml_dtypes.bfloat16)
    Wk = np.asarray(Wk, dtype=np.float32).astype(ml_dtypes.bfloat16)
    Wv = np.asarray(Wv, dtype=np.float32).astype(ml_dtypes.bfloat16)
    Wo = np.ascontiguousarray(
        np.asarray(Wo, dtype=np.float32).astype(ml_dtypes.bfloat16))
    bq = np.asarray(bq, dtype=np.float32)
    bk = np.asarray(bk, dtype=np.float32)
    bv = np.asarray(bv, dtype=np.float32)
    bo = np.ascontiguousarray(np.asarray(bo, dtype=np.float32))

    x_bf = x.astype(ml_dtypes.bfloat16)
    in_maps = []
    for c in range(N_CORES):
        b, hh = CORE_BH[c]
        hs = slice(hh * HL, (hh + 1) * HL)
        m = {
            "x": np.ascontiguousarray(x_bf[b]),
            "wq": np.ascontiguousarray(Wq[hs]),
            "wk": np.ascontiguousarray(Wk[hs]),
            "wv": np.ascontiguousarray(Wv[hs]),
            "wo": Wo,
            "bq": np.ascontiguousarray(bq[hs]),
            "bk": np.ascontiguousarray(bk[hs]),
            "bv": np.ascontiguousarray(bv[hs]),
            "bo": bo,
            "maskt": mt,
        }
        in_maps.append(m)

    res = run_bass_kernel_spmd(
        nc, in_maps, core_ids=list(range(N_CORES)),
        trace=os.environ.get("BASS_KERNEL_TRACE", "0") == "1")
    global LAST_RESULT
    LAST_RESULT = res

    out = np.empty((B, S, D), dtype=np.float32)
    for c in range(N_CORES):
        b, hh = CORE_BH[c]
        sl = slice(hh * NJ * ST, (hh + 1) * NJ * ST)
        out[b, sl, :] = res.results[c]["out"][sl]
    return out
